# revision 26
# baseline (speedup 1.0000x reference)
"""Trainium2 Bass kernel for the DeformationGraph problem.

Math: the reference computes, per batch b and vertex v,
    out[b,v,k] = sum_c W[v,c] * ( sum_d (X[b,v,d]-center[b,c,d]) * R[b,c,k,d]
                                  + center[b,c,k] + V_nodes[b,c,k] )
which factors into a vertex-independent per-node affine map:
    t[b,c,k]   = center[b,c,k] + V_nodes[b,c,k] - sum_d center[b,c,d]*R[b,c,k,d]
    out[b,v,k] = sum_d X[b,v,d] * (W @ R[..,k,d])[v]  +  (W @ t[..,k])[v]
i.e. one (V,C)@(C,48) matmul Y = W @ G, then a per-vertex contraction of Y
with [X,1].  The big tensors (W: 32MB, X, out) are sharded over the vertex
dimension across the 8 cores; G is replicated.

Layout: the 48 live Y rows sit at partitions j = d*16 + (k*4 + b), d in
0..3 (d==3 = translation/ones slot), rows 12..15 of each 16-block zero.
The 16-stride makes both halves of the d-reduction 32-aligned, which the
engines need, while keeping xd a single DMA.

fp32 matmul on TRN2 runs in LOW_HIGH dual-pass mode (~5x slower), so the
matmul uses the exact-enough 3-term bf16 split:
    W @ G ~= Wh@Gh + Wl@Gh + Wh@Gl     (Wh=bf16(W), Wl=bf16(W-Wh), ...)
measured end-to-end error vs the fp32 reference: ~4e-6 absmax.

The contraction dim C=160 splits into an A part (c 0..127, K=128) and a B
part (c 128..159, K=32).  The three B-part terms are packed into one K=96
matmul by stacking [WhB; WhB; WlB] against [GhB; GlB; GhB] host-side.

Reduction: engine 2-input ops need equal base partitions for SBUF+SBUF
pairs but allow arbitrary bases for mixed PSUM+SBUF pairs, so:
    DVE   p (64,n) PSUM  = y * xd
    ACT   q (32,n) SBUF  = copy p[32:64]
    DVE   a32 (32,n) SBUF = p[0:32] + q          (d0+d2 | d1+d3)
and the last level runs on the DMA engines (CCE add at the DRAM dest):
    DMA   outT[:, m]  = a32[0:12]   (HWDGE store)
    DMA   outT[:, m] += a32[16:28]  (SWDGE accumulate, dep-chained)

DMA macro chunks ramp up so compute starts early, then amortize the
~0.7us per-DMA sequencer issue cost; compute runs in 512-wide sub-chunks
(PSUM budget); a ~3.5us dummy-matmul warmup runs during the first DMAs to
lift the PE out of its cold 1.2GHz HAM state.
"""

import numpy as np
import ml_dtypes

import concourse.mybir as mybir
import concourse.tile as tile
from concourse import bacc
from concourse.bass_utils import run_bass_kernel_spmd
from concourse.tile_rust import add_dep_helper

B, V, C = 4, 50000, 160
N_CORES = 8
VS = V // N_CORES            # 6250 vertices per core
VSP = 6272                   # padded vertex shard
MACROS = [512, 1024, 2048, 2048, 640]
SUB = 512
F32 = mybir.dt.float32
BF16 = mybir.dt.bfloat16
NPBF16 = ml_dtypes.bfloat16


def _build_bass():
    nc = bacc.Bacc()

    wha_d = nc.dram_tensor("wha", [128, VSP], BF16, kind="ExternalInput")
    wla_d = nc.dram_tensor("wla", [128, VSP], BF16, kind="ExternalInput")
    wb_d = nc.dram_tensor("wb", [96, VSP], BF16, kind="ExternalInput")
    xd_d = nc.dram_tensor("xd", [64, VSP], F32, kind="ExternalInput")
    gh0_d = nc.dram_tensor("gh0", [128, 64], BF16, kind="ExternalInput")
    gl0_d = nc.dram_tensor("gl0", [128, 64], BF16, kind="ExternalInput")
    gbk_d = nc.dram_tensor("gbk", [96, 64], BF16, kind="ExternalInput")
    outT = nc.dram_tensor("outT", [12, VSP], F32, kind="ExternalOutput")

    with tile.TileContext(nc) as tc:
        with (
            tc.tile_pool(name="gpool", bufs=1) as gpool,
            tc.tile_pool(name="wpool", bufs=3) as wpool,
            tc.tile_pool(name="xpool", bufs=2) as xpool,
            tc.tile_pool(name="qpool", bufs=3) as qpool,
            tc.tile_pool(name="apool", bufs=2) as apool,
            tc.tile_pool(name="ypool", bufs=4, space="PSUM") as ypool,
            tc.tile_pool(name="ppool", bufs=2, space="PSUM") as ppool,
        ):
            gh0 = gpool.tile([128, 64], BF16)
            nc.sync.dma_start(out=gh0[:], in_=gh0_d[:])
            gl0 = gpool.tile([128, 64], BF16)
            nc.sync.dma_start(out=gl0[:], in_=gl0_d[:])
            gbk = gpool.tile([96, 64], BF16)
            nc.sync.dma_start(out=gbk[:], in_=gbk_d[:])

            # PE HAM warmup (output never read)
            wsc = gpool.tile([128, 512], BF16)
            nc.vector.memset(wsc[:], 0.0)
            ywarm = ypool.tile([64, 512], F32, tag="ywarm", bufs=1)
            for w in range(8):
                nc.tensor.matmul(ywarm[:, :], gh0[:], wsc[:, :],
                                 start=(w == 0), stop=(w == 7),
                                 skip_group_check=True)

            a32 = apool.tile([32, VSP], F32, bufs=1)
            xdt = xpool.tile([64, VSP], F32, bufs=1)
            nc.gpsimd.dma_start(out=xdt[:], in_=xd_d[:])

            m0 = 0
            for mn in MACROS:
                msl = slice(m0, m0 + mn)
                wha = wpool.tile([128, mn], BF16, tag="wha")
                nc.sync.dma_start(out=wha[:], in_=wha_d[:, msl])
                wla = wpool.tile([128, mn], BF16, tag="wla")
                nc.sync.dma_start(out=wla[:], in_=wla_d[:, msl])
                bpk = wpool.tile([96, mn], BF16, tag="bpk")
                nc.sync.dma_start(out=bpk[:], in_=wb_d[:, msl])

                for u0 in range(0, mn, SUB):
                    n = min(SUB, mn - u0)
                    us = slice(u0, u0 + n)
                    y = ypool.tile([64, n], F32, tag="y")
                    for h in range(0, n, 512):
                        hs = slice(u0 + h, u0 + min(h + 512, n))
                        ys = slice(h, min(h + 512, n))
                        nc.tensor.matmul(y[:, ys], gh0[:], wha[:, hs],
                                         start=True, stop=False)
                        nc.tensor.matmul(y[:, ys], gh0[:], wla[:, hs],
                                         start=False, stop=False)
                        nc.tensor.matmul(y[:, ys], gl0[:], wha[:, hs],
                                         start=False, stop=False)
                        nc.tensor.matmul(y[:, ys], gbk[:], bpk[:, hs],
                                         start=False, stop=True)

                    p = ppool.tile([64, n], F32, tag="p")
                    nc.vector.tensor_mul(out=p[:], in0=y[:], in1=xdt[:, m0 + u0:m0 + u0 + n])
                    q = qpool.tile([32, n], F32, tag="q")
                    nc.scalar.copy(out=q[:], in_=p[32:64, :])
                    nc.vector.tensor_add(out=a32[:, m0 + u0:m0 + u0 + n], in0=p[0:32, :],
                                         in1=q[:])

                d0 = nc.gpsimd.dma_start(out=outT[:, msl],
                                          in_=a32[0:12, msl])
                d1 = nc.gpsimd.dma_start(out=outT[:, msl],
                                         in_=a32[16:28, msl],
                                         accum_op=mybir.AluOpType.add)
                add_dep_helper(d1.ins, d0.ins,
                               reason="serialize DRAM accumulate after store")
                m0 += mn
    nc.finalize()
    return nc


_NC_CACHE = None


def _get_nc():
    global _NC_CACHE
    if _NC_CACHE is None:
        _NC_CACHE = _build_bass()
    return _NC_CACHE


def _host_prep(X, V_nodes, rot6d_nodes, W_nodes, idx_nn_to_nodes):
    """Small per-node math (B*C=640 rows) + shard/layout of the big tensors."""
    X = np.asarray(X, np.float32)
    Vn = np.asarray(V_nodes, np.float32)
    d6 = np.asarray(rot6d_nodes, np.float32)
    W = np.asarray(W_nodes, np.float32)
    idx = np.asarray(idx_nn_to_nodes).astype(np.int64)

    a1, a2 = d6[..., :3], d6[..., 3:]
    eps = np.float32(1e-8)
    n1 = np.sqrt(np.sum(a1 * a1, -1, keepdims=True, dtype=np.float32))
    b1 = a1 / np.maximum(n1, eps)
    dot = np.sum(b1 * a2, -1, keepdims=True, dtype=np.float32)
    a2p = a2 - dot * b1
    n2 = np.sqrt(np.sum(a2p * a2p, -1, keepdims=True, dtype=np.float32))
    b2 = a2p / np.maximum(n2, eps)
    b3 = np.cross(b1, b2)
    R = np.stack([b1, b2, b3], axis=-2).astype(np.float32)  # (B,C,3,3) [b,c,k,d]

    center = X[:, idx, :]                                   # (B,C,3)
    t = (center + Vn - np.einsum('bcd,bckd->bck', center, R)).astype(np.float32)

    # G columns at j = d*16 + k*4 + b; cols 12..15 of each block zero
    G = np.zeros((C, 64), np.float32)
    for d in range(4):
        for k in range(3):
            for b in range(B):
                j = d * 16 + k * 4 + b
                G[:, j] = R[b, :, k, d] if d < 3 else t[b, :, k]

    Gh = G.astype(NPBF16)
    Gl = (G - Gh.astype(np.float32)).astype(NPBF16)
    gh0 = np.ascontiguousarray(Gh[0:128])
    gl0 = np.ascontiguousarray(Gl[0:128])
    gbk = np.ascontiguousarray(
        np.concatenate([Gh[128:160], Gl[128:160], Gh[128:160]], axis=0))

    Wh = W.astype(NPBF16)
    Wl = (W - Wh.astype(np.float32)).astype(NPBF16)

    in_maps = []
    for i in range(N_CORES):
        vsl = slice(i * VS, (i + 1) * VS)
        wht = np.zeros((160, VSP), NPBF16)
        wht[:, :VS] = Wh[vsl].T
        wlt = np.zeros((160, VSP), NPBF16)
        wlt[:, :VS] = Wl[vsl].T
        wha = np.ascontiguousarray(wht[0:128])
        wla = np.ascontiguousarray(wlt[0:128])
        wb = np.ascontiguousarray(
            np.concatenate([wht[128:160], wht[128:160], wlt[128:160]], axis=0))
        # xd rows d*16 + k*4 + b: X[b,:,d] for d<3, ones for d==3
        xd = np.zeros((64, VSP), np.float32)
        for d in range(4):
            for k in range(3):
                for b in range(B):
                    r = d * 16 + k * 4 + b
                    xd[r, :VS] = X[b, vsl, d] if d < 3 else 1.0
        in_maps.append({"wha": wha, "wla": wla, "wb": wb, "xd": xd,
                        "gh0": gh0, "gl0": gl0, "gbk": gbk})
    return in_maps


def _gather(results):
    out = np.empty((B, V, 3), np.float32)
    for i, res in enumerate(results):
        oT = res["outT"]
        vsl = slice(i * VS, (i + 1) * VS)
        for k in range(3):
            for b in range(4):
                out[b, vsl, k] = oT[k * 4 + b, :VS]
    return out


def kernel(X, V_nodes, rot6d_nodes, W_nodes, idx_nn_to_nodes, **run_kwargs):
    in_maps = _host_prep(X, V_nodes, rot6d_nodes, W_nodes, idx_nn_to_nodes)
    res = run_bass_kernel_spmd(_get_nc(), in_maps,
                               core_ids=list(range(N_CORES)), **run_kwargs)
    out = _gather(res.results)
    kernel.last_run = res
    return out


# revision 27
# speedup vs baseline: 1.0973x; 1.0973x over previous
"""Trainium2 Bass kernel for the DeformationGraph problem.

Math: the reference computes, per batch b and vertex v,
    out[b,v,k] = sum_c W[v,c] * ( sum_d (X[b,v,d]-center[b,c,d]) * R[b,c,k,d]
                                  + center[b,c,k] + V_nodes[b,c,k] )
which factors into a vertex-independent per-node affine map:
    t[b,c,k]   = center[b,c,k] + V_nodes[b,c,k] - sum_d center[b,c,d]*R[b,c,k,d]
    out[b,v,k] = sum_d X[b,v,d] * (W @ R[..,k,d])[v]  +  (W @ t[..,k])[v]
i.e. one (V,C)@(C,48) matmul Y = W @ G, then a per-vertex contraction of Y
with [X,1].  The big tensors (W: 32MB, X, out) are sharded over the vertex
dimension across the 8 cores; G is replicated.

Layout: the 48 live Y rows sit at partitions j = d*16 + (k*4 + b), d in
0..3 (d==3 = translation/ones slot), rows 12..15 of each 16-block zero.
The 16-stride makes both halves of the d-reduction 32-aligned, which the
engines need, while keeping xd a single DMA.

fp32 matmul on TRN2 runs in LOW_HIGH dual-pass mode (~5x slower), so the
matmul uses the exact-enough 3-term bf16 split:
    W @ G ~= Wh@Gh + Wl@Gh + Wh@Gl     (Wh=bf16(W), Wl=bf16(W-Wh), ...)
measured end-to-end error vs the fp32 reference: ~4e-6 absmax.

The contraction dim C=160 splits into an A part (c 0..127, K=128) and a B
part (c 128..159, K=32).  The three B-part terms are packed into one K=96
matmul by stacking [WhB; WhB; WlB] against [GhB; GlB; GhB] host-side.

Reduction: engine 2-input ops need equal base partitions for SBUF+SBUF
pairs but allow arbitrary bases for mixed PSUM+SBUF pairs, so:
    DVE   p (64,n) PSUM  = y * xd
    ACT   q (32,n) SBUF  = copy p[32:64]
    DVE   a32 (32,n) SBUF = p[0:32] + q          (d0+d2 | d1+d3)
and the last level runs on the DMA engines (CCE add at the DRAM dest):
    DMA   outT[:, m]  = a32[0:12]   (HWDGE store)
    DMA   outT[:, m] += a32[16:28]  (SWDGE accumulate, dep-chained)

DMA macro chunks ramp up so compute starts early, then amortize the
~0.7us per-DMA sequencer issue cost; compute runs in 512-wide sub-chunks
(PSUM budget); a ~3.5us dummy-matmul warmup runs during the first DMAs to
lift the PE out of its cold 1.2GHz HAM state.
"""

import numpy as np
import ml_dtypes

import concourse.mybir as mybir
import concourse.tile as tile
from concourse import bacc
from concourse.bass_utils import run_bass_kernel_spmd
from concourse.tile_rust import add_dep_helper

B, V, C = 4, 50000, 160
N_CORES = 8
VS = V // N_CORES            # 6250 vertices per core
VSP = 6272                   # padded vertex shard
MACROS = [512, 1024, 2048, 2048, 640]
SUB = 512
F32 = mybir.dt.float32
BF16 = mybir.dt.bfloat16
NPBF16 = ml_dtypes.bfloat16


def _build_bass():
    nc = bacc.Bacc()

    wha_d = nc.dram_tensor("wha", [128, VSP], BF16, kind="ExternalInput")
    wla_d = nc.dram_tensor("wla", [128, VSP], BF16, kind="ExternalInput")
    wb_d = nc.dram_tensor("wb", [96, VSP], BF16, kind="ExternalInput")
    xd_d = nc.dram_tensor("xd", [64, VSP], F32, kind="ExternalInput")
    gh0_d = nc.dram_tensor("gh0", [128, 64], BF16, kind="ExternalInput")
    gl0_d = nc.dram_tensor("gl0", [128, 64], BF16, kind="ExternalInput")
    gbk_d = nc.dram_tensor("gbk", [96, 64], BF16, kind="ExternalInput")
    outT = nc.dram_tensor("outT", [12, VSP], F32, kind="ExternalOutput")

    with tile.TileContext(nc) as tc:
        with (
            tc.tile_pool(name="gpool", bufs=1) as gpool,
            tc.tile_pool(name="wpool", bufs=3) as wpool,
            tc.tile_pool(name="xpool", bufs=2) as xpool,
            tc.tile_pool(name="qpool", bufs=3) as qpool,
            tc.tile_pool(name="apool", bufs=2) as apool,
            tc.tile_pool(name="ypool", bufs=4, space="PSUM") as ypool,
            tc.tile_pool(name="ppool", bufs=2, space="PSUM") as ppool,
        ):
            gh0 = gpool.tile([128, 64], BF16)
            nc.sync.dma_start(out=gh0[:], in_=gh0_d[:])
            gl0 = gpool.tile([128, 64], BF16)
            nc.sync.dma_start(out=gl0[:], in_=gl0_d[:])
            gbk = gpool.tile([96, 64], BF16)
            nc.sync.dma_start(out=gbk[:], in_=gbk_d[:])

            # PE HAM warmup (output never read)
            wsc = gpool.tile([128, 512], BF16)
            nc.vector.memset(wsc[:], 0.0)
            ywarm = ypool.tile([64, 512], F32, tag="ywarm", bufs=1)
            for w in range(8):
                nc.tensor.matmul(ywarm[:, :], gh0[:], wsc[:, :],
                                 start=(w == 0), stop=(w == 7),
                                 skip_group_check=True)

            a32 = apool.tile([32, VSP], F32, bufs=1)

            m0 = 0
            for mn in MACROS:
                msl = slice(m0, m0 + mn)
                wha = wpool.tile([128, mn], BF16, tag="wha")
                nc.sync.dma_start(out=wha[:], in_=wha_d[:, msl])
                wla = wpool.tile([128, mn], BF16, tag="wla")
                nc.sync.dma_start(out=wla[:], in_=wla_d[:, msl])
                bpk = wpool.tile([96, mn], BF16, tag="bpk")
                nc.sync.dma_start(out=bpk[:], in_=wb_d[:, msl])
                xdt = xpool.tile([64, mn], F32, tag="xdt", bufs=5)
                nc.gpsimd.dma_start(out=xdt[:], in_=xd_d[:, msl])

                for u0 in range(0, mn, SUB):
                    n = min(SUB, mn - u0)
                    us = slice(u0, u0 + n)
                    y = ypool.tile([64, n], F32, tag="y")
                    for h in range(0, n, 512):
                        hs = slice(u0 + h, u0 + min(h + 512, n))
                        ys = slice(h, min(h + 512, n))
                        nc.tensor.matmul(y[:, ys], gh0[:], wha[:, hs],
                                         start=True, stop=False)
                        nc.tensor.matmul(y[:, ys], gh0[:], wla[:, hs],
                                         start=False, stop=False)
                        nc.tensor.matmul(y[:, ys], gl0[:], wha[:, hs],
                                         start=False, stop=False)
                        nc.tensor.matmul(y[:, ys], gbk[:], bpk[:, hs],
                                         start=False, stop=True)

                    p = ppool.tile([64, n], F32, tag="p")
                    nc.vector.tensor_mul(out=p[:], in0=y[:], in1=xdt[:, us])
                    q = qpool.tile([32, n], F32, tag="q")
                    nc.scalar.copy(out=q[:], in_=p[32:64, :])
                    nc.vector.tensor_add(out=a32[:, m0 + u0:m0 + u0 + n], in0=p[0:32, :],
                                         in1=q[:])

                m0 += mn

            m0 = 0
            for mn in MACROS:
                msl = slice(m0, m0 + mn)
                d0 = nc.sync.dma_start(out=outT[:, msl], in_=a32[0:12, msl])
                d1 = nc.gpsimd.dma_start(out=outT[:, msl],
                                         in_=a32[16:28, msl],
                                         accum_op=mybir.AluOpType.add)
                add_dep_helper(d1.ins, d0.ins,
                               reason="serialize DRAM accumulate after store")
                m0 += mn
    nc.finalize()
    return nc


_NC_CACHE = None


def _get_nc():
    global _NC_CACHE
    if _NC_CACHE is None:
        _NC_CACHE = _build_bass()
    return _NC_CACHE


def _host_prep(X, V_nodes, rot6d_nodes, W_nodes, idx_nn_to_nodes):
    """Small per-node math (B*C=640 rows) + shard/layout of the big tensors."""
    X = np.asarray(X, np.float32)
    Vn = np.asarray(V_nodes, np.float32)
    d6 = np.asarray(rot6d_nodes, np.float32)
    W = np.asarray(W_nodes, np.float32)
    idx = np.asarray(idx_nn_to_nodes).astype(np.int64)

    a1, a2 = d6[..., :3], d6[..., 3:]
    eps = np.float32(1e-8)
    n1 = np.sqrt(np.sum(a1 * a1, -1, keepdims=True, dtype=np.float32))
    b1 = a1 / np.maximum(n1, eps)
    dot = np.sum(b1 * a2, -1, keepdims=True, dtype=np.float32)
    a2p = a2 - dot * b1
    n2 = np.sqrt(np.sum(a2p * a2p, -1, keepdims=True, dtype=np.float32))
    b2 = a2p / np.maximum(n2, eps)
    b3 = np.cross(b1, b2)
    R = np.stack([b1, b2, b3], axis=-2).astype(np.float32)  # (B,C,3,3) [b,c,k,d]

    center = X[:, idx, :]                                   # (B,C,3)
    t = (center + Vn - np.einsum('bcd,bckd->bck', center, R)).astype(np.float32)

    # G columns at j = d*16 + k*4 + b; cols 12..15 of each block zero
    G = np.zeros((C, 64), np.float32)
    for d in range(4):
        for k in range(3):
            for b in range(B):
                j = d * 16 + k * 4 + b
                G[:, j] = R[b, :, k, d] if d < 3 else t[b, :, k]

    Gh = G.astype(NPBF16)
    Gl = (G - Gh.astype(np.float32)).astype(NPBF16)
    gh0 = np.ascontiguousarray(Gh[0:128])
    gl0 = np.ascontiguousarray(Gl[0:128])
    gbk = np.ascontiguousarray(
        np.concatenate([Gh[128:160], Gl[128:160], Gh[128:160]], axis=0))

    Wh = W.astype(NPBF16)
    Wl = (W - Wh.astype(np.float32)).astype(NPBF16)

    in_maps = []
    for i in range(N_CORES):
        vsl = slice(i * VS, (i + 1) * VS)
        wht = np.zeros((160, VSP), NPBF16)
        wht[:, :VS] = Wh[vsl].T
        wlt = np.zeros((160, VSP), NPBF16)
        wlt[:, :VS] = Wl[vsl].T
        wha = np.ascontiguousarray(wht[0:128])
        wla = np.ascontiguousarray(wlt[0:128])
        wb = np.ascontiguousarray(
            np.concatenate([wht[128:160], wht[128:160], wlt[128:160]], axis=0))
        # xd rows d*16 + k*4 + b: X[b,:,d] for d<3, ones for d==3
        xd = np.zeros((64, VSP), np.float32)
        for d in range(4):
            for k in range(3):
                for b in range(B):
                    r = d * 16 + k * 4 + b
                    xd[r, :VS] = X[b, vsl, d] if d < 3 else 1.0
        in_maps.append({"wha": wha, "wla": wla, "wb": wb, "xd": xd,
                        "gh0": gh0, "gl0": gl0, "gbk": gbk})
    return in_maps


def _gather(results):
    out = np.empty((B, V, 3), np.float32)
    for i, res in enumerate(results):
        oT = res["outT"]
        vsl = slice(i * VS, (i + 1) * VS)
        for k in range(3):
            for b in range(4):
                out[b, vsl, k] = oT[k * 4 + b, :VS]
    return out


def kernel(X, V_nodes, rot6d_nodes, W_nodes, idx_nn_to_nodes, **run_kwargs):
    in_maps = _host_prep(X, V_nodes, rot6d_nodes, W_nodes, idx_nn_to_nodes)
    res = run_bass_kernel_spmd(_get_nc(), in_maps,
                               core_ids=list(range(N_CORES)), **run_kwargs)
    out = _gather(res.results)
    kernel.last_run = res
    return out


# revision 28
# speedup vs baseline: 1.1223x; 1.0227x over previous
"""Trainium2 Bass kernel for the DeformationGraph problem.

Math: the reference computes, per batch b and vertex v,
    out[b,v,k] = sum_c W[v,c] * ( sum_d (X[b,v,d]-center[b,c,d]) * R[b,c,k,d]
                                  + center[b,c,k] + V_nodes[b,c,k] )
which factors into a vertex-independent per-node affine map:
    t[b,c,k]   = center[b,c,k] + V_nodes[b,c,k] - sum_d center[b,c,d]*R[b,c,k,d]
    out[b,v,k] = sum_d X[b,v,d] * (W @ R[..,k,d])[v]  +  (W @ t[..,k])[v]
i.e. one (V,C)@(C,48) matmul Y = W @ G, then a per-vertex contraction of Y
with [X,1].  The big tensors (W: 32MB, X, out) are sharded over the vertex
dimension across the 8 cores; G is replicated.

Layout: the 48 live Y rows sit at partitions j = d*16 + (k*4 + b), d in
0..3 (d==3 = translation/ones slot), rows 12..15 of each 16-block zero.
The 16-stride makes both halves of the d-reduction 32-aligned, which the
engines need, while keeping xd a single DMA.

fp32 matmul on TRN2 runs in LOW_HIGH dual-pass mode (~5x slower), so the
matmul uses the exact-enough 3-term bf16 split:
    W @ G ~= Wh@Gh + Wl@Gh + Wh@Gl     (Wh=bf16(W), Wl=bf16(W-Wh), ...)
measured end-to-end error vs the fp32 reference: ~4e-6 absmax.

The contraction dim C=160 splits into an A part (c 0..127, K=128) and a B
part (c 128..159, K=32).  The three B-part terms are packed into one K=96
matmul by stacking [WhB; WhB; WlB] against [GhB; GlB; GhB] host-side.

Reduction: engine 2-input ops need equal base partitions for SBUF+SBUF
pairs but allow arbitrary bases for mixed PSUM+SBUF pairs, so:
    DVE   p (64,n) PSUM  = y * xd
    ACT   q (32,n) SBUF  = copy p[32:64]
    DVE   a32 (32,n) SBUF = p[0:32] + q          (d0+d2 | d1+d3)
and the last level runs on the DMA engines (CCE add at the DRAM dest):
    DMA   outT[:, m]  = a32[0:12]   (HWDGE store)
    DMA   outT[:, m] += a32[16:28]  (SWDGE accumulate, dep-chained)

DMA macro chunks ramp up so compute starts early, then amortize the
~0.7us per-DMA sequencer issue cost; compute runs in 512-wide sub-chunks
(PSUM budget); a ~3.5us dummy-matmul warmup runs during the first DMAs to
lift the PE out of its cold 1.2GHz HAM state.
"""

import numpy as np
import ml_dtypes

import concourse.mybir as mybir
import concourse.tile as tile
from concourse import bacc
from concourse.bass_utils import run_bass_kernel_spmd
from concourse.tile_rust import add_dep_helper

B, V, C = 4, 50000, 160
N_CORES = 8
VS = V // N_CORES            # 6250 vertices per core
VSP = 6272                   # padded vertex shard
MACROS = [512, 1024, 2048, 2048, 640]
SUB = 512
F32 = mybir.dt.float32
BF16 = mybir.dt.bfloat16
NPBF16 = ml_dtypes.bfloat16


def _build_bass():
    nc = bacc.Bacc()

    wha_d = nc.dram_tensor("wha", [128, VSP], BF16, kind="ExternalInput")
    wla_d = nc.dram_tensor("wla", [128, VSP], BF16, kind="ExternalInput")
    wb_d = nc.dram_tensor("wb", [96, VSP], BF16, kind="ExternalInput")
    xd_d = nc.dram_tensor("xd", [64, VSP], F32, kind="ExternalInput")
    gh0_d = nc.dram_tensor("gh0", [128, 64], BF16, kind="ExternalInput")
    gl0_d = nc.dram_tensor("gl0", [128, 64], BF16, kind="ExternalInput")
    gbk_d = nc.dram_tensor("gbk", [96, 64], BF16, kind="ExternalInput")
    outT = nc.dram_tensor("outT", [12, VSP], F32, kind="ExternalOutput")

    with tile.TileContext(nc) as tc:
        with (
            tc.tile_pool(name="gpool", bufs=1) as gpool,
            tc.tile_pool(name="wpool", bufs=3) as wpool,
            tc.tile_pool(name="xpool", bufs=2) as xpool,
            tc.tile_pool(name="qpool", bufs=3) as qpool,
            tc.tile_pool(name="apool", bufs=2) as apool,
            tc.tile_pool(name="ypool", bufs=4, space="PSUM") as ypool,
            tc.tile_pool(name="ppool", bufs=2, space="PSUM") as ppool,
        ):
            gh0 = gpool.tile([128, 64], BF16)
            nc.sync.dma_start(out=gh0[:], in_=gh0_d[:])
            gl0 = gpool.tile([128, 64], BF16)
            nc.sync.dma_start(out=gl0[:], in_=gl0_d[:])
            gbk = gpool.tile([96, 64], BF16)
            nc.sync.dma_start(out=gbk[:], in_=gbk_d[:])

            # PE HAM warmup (output never read)
            wsc = gpool.tile([128, 512], BF16)
            nc.vector.memset(wsc[:], 0.0)
            ywarm = ypool.tile([64, 512], F32, tag="ywarm", bufs=1)
            for w in range(8):
                nc.tensor.matmul(ywarm[:, :], gh0[:], wsc[:, :],
                                 start=(w == 0), stop=(w == 7),
                                 skip_group_check=True)

            a32 = apool.tile([32, VSP], F32, bufs=1)

            m0 = 0
            for mn in MACROS:
                msl = slice(m0, m0 + mn)
                wha = wpool.tile([128, mn], BF16, tag="wha")
                nc.sync.dma_start(out=wha[:], in_=wha_d[:, msl])
                wla = wpool.tile([128, mn], BF16, tag="wla")
                nc.sync.dma_start(out=wla[:], in_=wla_d[:, msl])
                bpk = wpool.tile([96, mn], BF16, tag="bpk")
                nc.sync.dma_start(out=bpk[:], in_=wb_d[:, msl])
                xdt = xpool.tile([64, mn], F32, tag="xdt", bufs=5)
                nc.gpsimd.dma_start(out=xdt[:], in_=xd_d[:, msl])

                # process sub-chunks in PAIRS: the even sub-chunk's matmul
                # group runs in PE column-group 0 (PSUM partitions 0:64),
                # the odd one's in column-group 64 — interleaved issue makes
                # the two groups stream concurrently through the array
                # (M=64 uses only half the PE columns otherwise).
                for u0 in range(0, mn, 2 * SUB):
                    n1 = min(SUB, mn - u0)
                    n2 = min(SUB, mn - u0 - n1)
                    u1 = u0 + n1
                    y = ypool.tile([128, SUB], F32, tag="y")
                    terms = ((gh0, wha), (gh0, wla), (gl0, wha), (gbk, bpk))
                    for t, (g, w) in enumerate(terms):
                        nc.tensor.matmul(y[0:64, 0:n1], g[:],
                                         w[:, u0:u0 + n1],
                                         start=(t == 0), stop=(t == 3),
                                         skip_group_check=True)
                        if n2:
                            nc.tensor.matmul(y[64:128, 0:n2], g[:],
                                             w[:, u1:u1 + n2],
                                             start=(t == 0), stop=(t == 3),
                                             skip_group_check=True)

                    p = ppool.tile([128, SUB], F32, tag="p")
                    nc.vector.tensor_mul(out=p[0:64, 0:n1], in0=y[0:64, 0:n1],
                                         in1=xdt[:, u0:u0 + n1])
                    q = qpool.tile([32, n1], F32, tag="q")
                    nc.scalar.copy(out=q[:], in_=p[32:64, 0:n1])
                    nc.vector.tensor_add(out=a32[:, m0 + u0:m0 + u0 + n1],
                                         in0=p[0:32, 0:n1], in1=q[:])
                    if n2:
                        nc.vector.tensor_mul(out=p[64:128, 0:n2],
                                             in0=y[64:128, 0:n2],
                                             in1=xdt[:, u1:u1 + n2])
                        q2 = qpool.tile([32, n2], F32, tag="q2")
                        nc.scalar.copy(out=q2[:], in_=p[96:128, 0:n2])
                        nc.vector.tensor_add(out=a32[:, m0 + u1:m0 + u1 + n2],
                                             in0=p[64:96, 0:n2], in1=q2[:])

                m0 += mn

            m0 = 0
            for mn in MACROS:
                msl = slice(m0, m0 + mn)
                d0 = nc.sync.dma_start(out=outT[:, msl], in_=a32[0:12, msl])
                d1 = nc.gpsimd.dma_start(out=outT[:, msl],
                                         in_=a32[16:28, msl],
                                         accum_op=mybir.AluOpType.add)
                add_dep_helper(d1.ins, d0.ins,
                               reason="serialize DRAM accumulate after store")
                m0 += mn
    nc.finalize()
    return nc


_NC_CACHE = None


def _get_nc():
    global _NC_CACHE
    if _NC_CACHE is None:
        _NC_CACHE = _build_bass()
    return _NC_CACHE


def _host_prep(X, V_nodes, rot6d_nodes, W_nodes, idx_nn_to_nodes):
    """Small per-node math (B*C=640 rows) + shard/layout of the big tensors."""
    X = np.asarray(X, np.float32)
    Vn = np.asarray(V_nodes, np.float32)
    d6 = np.asarray(rot6d_nodes, np.float32)
    W = np.asarray(W_nodes, np.float32)
    idx = np.asarray(idx_nn_to_nodes).astype(np.int64)

    a1, a2 = d6[..., :3], d6[..., 3:]
    eps = np.float32(1e-8)
    n1 = np.sqrt(np.sum(a1 * a1, -1, keepdims=True, dtype=np.float32))
    b1 = a1 / np.maximum(n1, eps)
    dot = np.sum(b1 * a2, -1, keepdims=True, dtype=np.float32)
    a2p = a2 - dot * b1
    n2 = np.sqrt(np.sum(a2p * a2p, -1, keepdims=True, dtype=np.float32))
    b2 = a2p / np.maximum(n2, eps)
    b3 = np.cross(b1, b2)
    R = np.stack([b1, b2, b3], axis=-2).astype(np.float32)  # (B,C,3,3) [b,c,k,d]

    center = X[:, idx, :]                                   # (B,C,3)
    t = (center + Vn - np.einsum('bcd,bckd->bck', center, R)).astype(np.float32)

    # G columns at j = d*16 + k*4 + b; cols 12..15 of each block zero
    G = np.zeros((C, 64), np.float32)
    for d in range(4):
        for k in range(3):
            for b in range(B):
                j = d * 16 + k * 4 + b
                G[:, j] = R[b, :, k, d] if d < 3 else t[b, :, k]

    Gh = G.astype(NPBF16)
    Gl = (G - Gh.astype(np.float32)).astype(NPBF16)
    gh0 = np.ascontiguousarray(Gh[0:128])
    gl0 = np.ascontiguousarray(Gl[0:128])
    gbk = np.ascontiguousarray(
        np.concatenate([Gh[128:160], Gl[128:160], Gh[128:160]], axis=0))

    Wh = W.astype(NPBF16)
    Wl = (W - Wh.astype(np.float32)).astype(NPBF16)

    in_maps = []
    for i in range(N_CORES):
        vsl = slice(i * VS, (i + 1) * VS)
        wht = np.zeros((160, VSP), NPBF16)
        wht[:, :VS] = Wh[vsl].T
        wlt = np.zeros((160, VSP), NPBF16)
        wlt[:, :VS] = Wl[vsl].T
        wha = np.ascontiguousarray(wht[0:128])
        wla = np.ascontiguousarray(wlt[0:128])
        wb = np.ascontiguousarray(
            np.concatenate([wht[128:160], wht[128:160], wlt[128:160]], axis=0))
        # xd rows d*16 + k*4 + b: X[b,:,d] for d<3, ones for d==3
        xd = np.zeros((64, VSP), np.float32)
        for d in range(4):
            for k in range(3):
                for b in range(B):
                    r = d * 16 + k * 4 + b
                    xd[r, :VS] = X[b, vsl, d] if d < 3 else 1.0
        in_maps.append({"wha": wha, "wla": wla, "wb": wb, "xd": xd,
                        "gh0": gh0, "gl0": gl0, "gbk": gbk})
    return in_maps


def _gather(results):
    out = np.empty((B, V, 3), np.float32)
    for i, res in enumerate(results):
        oT = res["outT"]
        vsl = slice(i * VS, (i + 1) * VS)
        for k in range(3):
            for b in range(4):
                out[b, vsl, k] = oT[k * 4 + b, :VS]
    return out


def kernel(X, V_nodes, rot6d_nodes, W_nodes, idx_nn_to_nodes, **run_kwargs):
    in_maps = _host_prep(X, V_nodes, rot6d_nodes, W_nodes, idx_nn_to_nodes)
    res = run_bass_kernel_spmd(_get_nc(), in_maps,
                               core_ids=list(range(N_CORES)), **run_kwargs)
    out = _gather(res.results)
    kernel.last_run = res
    return out
